# revision 26
# baseline (speedup 1.0000x reference)
"""ArcMargin head (ArcFace) distributed over 8 TRN2 NeuronCores.

Strategy (classification / tensor parallel), v5 — no on-chip transposes:
  - weight [C, D] sharded along C (12500 classes/core, padded to 12544);
    embeddings + labels replicated.
  - The host uploads the weight shard twice in bf16 (same bytes as one f32
    copy): pre-TRANSPOSED [D, CSP] for the matmul lhsT, natural [CSP, D] for
    the class-norm computation and the per-label row gather.  Embeddings are
    likewise uploaded bf16 both ways ([B, D] for row norms / target dots and
    [D, B] as the raw matmul rhs), so the TensorEngine does no layout work.
  - The device computes TRANSPOSED logits out[c, b] = 64 * (w_c . e_hat_b):
    lhsT = wT tile (classes stationary, 128 at a time), rhs = raw embT.
    Classes sit on PSUM partitions, so BOTH normalizations fold into PSUM
    evacuation: 1/||w_c|| is a per-partition scalar, and 64/||e_b|| is a
    per-column tensor `renb` [128, B].  renb is built once: row norms ->
    erec [128,16] -> bf16 -> DMA to a DRAM scratch through a diagonal
    strided view -> DMA back as a [16, B] block-diagonal matrix -> K=16
    ones-matmul broadcasts it across all 128 partitions.  Output is bf16
    (halves the dominant HBM traffic; rel-err budget 2e-2 >> bf16 noise).
  - ArcFace margin: only the single target element per row changes.  Target
    cosines come from an indirect row gather of weight[labels] plus a fused
    multiply-accumulate dot; phi values leave in a tiny [128, 16] tensor and
    are placed into the full output during the host unshard (all math on
    device; the host only does indexing).
"""

import math
import sys

import numpy as np
import ml_dtypes

for _p in ("/opt/trn_rl_repo",):
    if _p not in sys.path:
        sys.path.append(_p)

import concourse.bass as bass
import concourse.tile as tile
from concourse import bacc
from concourse import mybir
from concourse.bass_utils import run_bass_kernel_spmd

SCALE = 64.0
MARGIN = 0.5
COS_M = math.cos(MARGIN)
SIN_M = math.sin(MARGIN)
TH = math.cos(math.pi - MARGIN)
MM = math.sin(math.pi - MARGIN) * MARGIN

B, D, C = 2048, 512, 100000
N_CORES = 8
CS = C // N_CORES          # 12500 real classes per core
CSP = 12544                # padded classes per core (98 * 128)
NJ = CSP // 128            # 98 class chunks
CB = 1792                  # weight-block width (7 blocks x 14 chunks)
NBLK = CSP // CB           # 7
JPB = CB // 128            # 14 chunks per block
OOB = 1 << 30              # gather offset sentinel for "not my row"


NPBF = ml_dtypes.bfloat16

F32 = mybir.dt.float32
BF16 = mybir.dt.bfloat16
I32 = mybir.dt.int32
AF = mybir.ActivationFunctionType
ALU = mybir.AluOpType


def build_program(b=B, d=D, csp=CSP):
    """Build the (SPMD-uniform) single-core Bass program."""
    mb = b // 128          # 16 batch row-chunks
    kc = d // 128          # 4 contraction chunks
    nc = bacc.Bacc()

    emb_d = nc.declare_dram_parameter("emb", [b, d], BF16, isOutput=False)
    embt_d = nc.declare_dram_parameter("embt", [d, b], BF16, isOutput=False)
    wt_d = nc.declare_dram_parameter("wt", [d, csp], BF16, isOutput=False)
    wn_d = nc.declare_dram_parameter("wn", [csp, d], BF16, isOutput=False)
    goff_d = nc.declare_dram_parameter("goff", [128, mb], I32, isOutput=False)
    # flat transposed output [c * B + b]
    out_d = nc.declare_dram_parameter("out", [csp * b, 1], BF16, isOutput=True)
    tv_d = nc.declare_dram_parameter("tv", [128, mb], F32, isOutput=True)
    zd_d = nc.declare_dram_parameter("zd", [B, 1], BF16, isOutput=True)

    with tile.TileContext(nc) as tc:
        with (
            tc.tile_pool(name="const", bufs=1) as constp,
            tc.tile_pool(name="persist", bufs=1) as persist,
            tc.tile_pool(name="eld", bufs=1) as eldp,
            tc.tile_pool(name="wtp", bufs=3) as wtp,
            tc.tile_pool(name="wnp", bufs=3) as wnp,
            tc.tile_pool(name="scr", bufs=2) as scrp,
            tc.tile_pool(name="smp", bufs=4) as smp,
            tc.tile_pool(name="outp", bufs=4) as outp,
            tc.tile_pool(name="cpsum", bufs=4, space="PSUM") as cpsum,
        ):
            zb = constp.tile([128, 1], F32, tag="zb")
            nc.vector.memset(zb[:], 0.0)
            epsb = constp.tile([128, 1], F32, tag="epsb")
            nc.vector.memset(epsb[:], 1e-24)
            s2b = constp.tile([128, 1], F32, tag="s2b")
            nc.vector.memset(s2b[:], SCALE * SCALE)

            embt = persist.tile([128, kc, b], BF16)     # e^T raw (matmul rhs)
            re1 = persist.tile([1, b], BF16, tag="re1")
            renb = persist.tile([128, b], BF16)         # 64/||e_b|| bcast
            essq = persist.tile([128, mb], F32)         # per-row sum(e^2)
            erec = persist.tile([128, mb], F32)         # 64/||e_b||, natural
            erecb = persist.tile([128, mb], BF16)
            nsq = persist.tile([128, NJ], F32)          # per-class sum(w^2)
            nrm = persist.tile([128, NJ], F32)
            rn = persist.tile([128, NJ], F32)           # 1/||w_c||
            svec = persist.tile([128, mb], F32)         # 64*cos(target)
            tval = persist.tile([128, mb], F32)         # 64*phi / else-branch
            gofft = persist.tile([128, mb], I32)

            outv = out_d[:].rearrange("(c b) o -> c (b o)", b=b)  # [csp, b]
            # flatten-across-partitions views: write (p, q) -> q*128 + p,
            # read back the whole thing as one [1, B] row
            zd_w = zd_d[:].rearrange("(q p) o -> p (q o)", p=128)
            zd_r = zd_d[:].rearrange("(a bb) o -> a (bb o)", a=1)

            # ---------------- DMA helpers ----------------
            def wt_blk(blk):
                t = wtp.tile([128, kc, CB], BF16, tag="wt", name=f"wt_{blk}")
                nc.sync.dma_start(
                    out=t[:],
                    in_=wt_d[:, blk * CB:(blk + 1) * CB].rearrange(
                        "(k p) c -> p k c", p=128
                    ),
                )
                return t

            wn_tiles = {}

            def wn_g(g):
                r0 = g * 512
                ng = min(4, NJ - g * 4)
                t = wnp.tile([128, 4, d], BF16, tag="wn", name=f"wn_{g}")
                nc.sync.dma_start(
                    out=t[:, :ng, :],
                    in_=wn_d[r0:r0 + ng * 128, :].rearrange(
                        "(g2 p) dd -> p g2 dd", p=128
                    ),
                )
                wn_tiles[g] = t

            # ---------------- compute helpers ----------------
            def wnorm_chunk(c):
                sq = scrp.tile([128, d], BF16, tag="sqw")
                nc.scalar.activation(
                    out=sq[:], in_=wn_tiles[c // 4][:, c % 4, :], func=AF.Square,
                    bias=zb[:], accum_out=nsq[:, c:c + 1],
                )

            def rn_fin(g):
                s0 = g * 4
                s1 = min(s0 + 4, NJ)
                nc.scalar.activation(
                    out=nrm[:, s0:s1], in_=nsq[:, s0:s1], func=AF.Sqrt, bias=epsb[:]
                )
                nc.vector.reciprocal(out=rn[:, s0:s1], in_=nrm[:, s0:s1])

            def phase_wsel(m, egn):
                # gather this chunk's owned target weight rows from the shard
                wsld = scrp.tile([128, d], BF16, tag="wsld", name=f"ws_{m}")
                nc.gpsimd.indirect_dma_start(
                    out=wsld[:],
                    out_offset=None,
                    in_=wn_d[:],
                    in_offset=bass.IndirectOffsetOnAxis(
                        ap=gofft[:, m:m + 1], axis=0
                    ),
                    bounds_check=csp - 1,
                    oob_is_err=False,
                )
                sq = scrp.tile([128, d], BF16, tag="sqs")
                ssq = smp.tile([128, 1], F32, tag="ssqs")
                nc.scalar.activation(
                    out=sq[:], in_=wsld[:], func=AF.Square, bias=zb[:],
                    accum_out=ssq[:],
                )
                nrm_s = smp.tile([128, 1], F32, tag="nrms")
                nc.scalar.activation(out=nrm_s[:], in_=ssq[:], func=AF.Sqrt, bias=epsb[:])
                rec = smp.tile([128, 1], F32, tag="recs")
                nc.vector.reciprocal(out=rec[:], in_=nrm_s[:])
                # raw dot sum(wsld * e_raw); scale by 1/||w|| and 64/||e|| after
                ttr = scrp.tile([128, d], BF16, tag="ttr")
                dotm = smp.tile([128, 1], F32, tag="dotm")
                nc.vector.scalar_tensor_tensor(
                    out=ttr[:], in0=wsld[:], scalar=1.0, in1=egn,
                    op0=ALU.mult, op1=ALU.mult, accum_out=dotm[:],
                )
                dw = smp.tile([128, 1], F32, tag="dw")
                nc.vector.tensor_tensor(
                    out=dw[:], in0=dotm[:], in1=rec[:], op=ALU.mult
                )
                nc.vector.tensor_tensor(
                    out=svec[:, m:m + 1], in0=dw[:], in1=erec[:, m:m + 1],
                    op=ALU.mult,
                )

            def phi_block():
                s2 = smp.tile([128, mb], F32, tag="s2")
                nc.scalar.activation(
                    out=s2[:], in_=svec[:], func=AF.Square, bias=zb[:]
                )
                rl = smp.tile([128, mb], F32, tag="rl")
                nc.scalar.activation(
                    out=rl[:], in_=s2[:], func=AF.Relu, bias=s2b[:], scale=-1.0
                )
                sn = smp.tile([128, mb], F32, tag="sn")
                nc.scalar.activation(out=sn[:], in_=rl[:], func=AF.Sqrt, bias=zb[:])
                pc = smp.tile([128, mb], F32, tag="pc")
                nc.vector.tensor_scalar_mul(out=pc[:], in0=svec[:], scalar1=COS_M)
                smt = smp.tile([128, mb], F32, tag="smt")
                nc.vector.tensor_scalar_mul(out=smt[:], in0=sn[:], scalar1=SIN_M)
                ph = smp.tile([128, mb], F32, tag="ph")
                nc.vector.tensor_tensor(
                    out=ph[:], in0=pc[:], in1=smt[:], op=ALU.subtract
                )
                eb = smp.tile([128, mb], F32, tag="eb")
                nc.vector.tensor_scalar_add(
                    out=eb[:], in0=svec[:], scalar1=-SCALE * MM
                )
                mk = smp.tile([128, mb], mybir.dt.uint8, tag="mk")
                nc.vector.tensor_scalar(
                    out=mk[:], in0=svec[:], scalar1=SCALE * TH, scalar2=None,
                    op0=ALU.is_gt,
                )
                nc.vector.select(out=tval[:], mask=mk[:], on_true=ph[:], on_false=eb[:])
                nc.sync.dma_start(out=tv_d[:], in_=tval[:])

            # ---------------- prologue (DMA order = ring order) ----------------
            # natural-layout embeddings on the gpsimd (SWDGE) queue: they gate
            # the renb chain, and cross-queue DMA deps act as per-queue
            # barriers, so keep this chain off the busy sync rings entirely
            egn = eldp.tile([128, mb // 4, 4, d], BF16, tag="egn")
            for g in range(4):
                nc.gpsimd.dma_start(
                    out=egn[:, g, :, :],
                    in_=emb_d[g * 512:(g + 1) * 512, :].rearrange(
                        "(g2 p) dd -> p g2 dd", p=128
                    ),
                )
            # transposed raw embeddings: b-halves
            for hh in range(2):
                nc.sync.dma_start(
                    out=embt[:, :, hh * 1024:(hh + 1) * 1024],
                    in_=embt_d[:, hh * 1024:(hh + 1) * 1024].rearrange(
                        "(k p) c -> p k c", p=128
                    ),
                )
            # split first weight block: j0/j1 lhsT available early
            wt0a = wtp.tile([128, kc, 256], BF16, tag="wt0a")
            nc.sync.dma_start(
                out=wt0a[:],
                in_=wt_d[:, 0:256].rearrange("(k p) c -> p k c", p=128),
            )
            wn_g(0)
            wt0b = wtp.tile([128, kc, CB - 256], BF16, tag="wt0b")
            nc.sync.dma_start(
                out=wt0b[:],
                in_=wt_d[:, 256:CB].rearrange("(k p) c -> p k c", p=128),
            )
            wn_g(1)
            nc.sync.dma_start(out=gofft[:], in_=goff_d[:])

            # row norms of e: squares alternate Scalar/Vector, accum into essq
            for m in range(mb):
                et = egn[:, m // 4, m % 4, :]
                if m % 2 == 0:
                    sq = scrp.tile([128, d], BF16, tag="sqe")
                    nc.scalar.activation(
                        out=sq[:], in_=et, func=AF.Square, bias=zb[:],
                        accum_out=essq[:, m:m + 1],
                    )
                else:
                    sq = scrp.tile([128, d], BF16, tag="sqe1")
                    nc.vector.scalar_tensor_tensor(
                        out=sq[:], in0=et, scalar=1.0, in1=et,
                        op0=ALU.mult, op1=ALU.mult, accum_out=essq[:, m:m + 1],
                    )
            # erec = 64/||e|| = 1/sqrt(essq/4096 + eps)
            enrm = smp.tile([128, mb], F32, tag="enrm")
            nc.scalar.activation(
                out=enrm[:], in_=essq[:], func=AF.Sqrt, bias=epsb[:],
                scale=1.0 / (SCALE * SCALE),
            )
            nc.vector.reciprocal(out=erec[:], in_=enrm[:])
            nc.vector.tensor_copy(out=erecb[:], in_=erec[:])
            # flatten round-trip: erecb [128,16] -> DRAM -> [1, B] row,
            # then Pool-engine partition broadcast -> renb [128, B].
            # All on the gpsimd queue; the PE is not involved at all.
            nc.gpsimd.dma_start(out=zd_w, in_=erecb[:])
            nc.gpsimd.dma_start(out=re1[:], in_=zd_r)
            nc.gpsimd.partition_broadcast(out_ap=renb[:], in_ap=re1[:])

            wdone = 0
            while wdone < 8:
                wnorm_chunk(wdone)
                wdone += 1
                if wdone % 4 == 0:
                    rn_fin(wdone // 4 - 1)

            # ---------------- main loop over class chunks ----------------
            cur_wt = None
            nxt_wt = None
            wsel_done = 0
            for j in range(NJ):
                blk, jj = divmod(j, JPB)
                if jj == 0:
                    if blk > 0:
                        cur_wt = nxt_wt
                    if blk < NBLK - 1:
                        nxt_wt = wt_blk(blk + 1)
                if j % 4 == 0:
                    g = j // 4 + 2
                    if g * 4 < NJ:
                        wn_g(g)
                while wdone < min(NJ, j + 9):
                    wnorm_chunk(wdone)
                    wdone += 1
                    if wdone % 4 == 0 or wdone == NJ:
                        rn_fin((wdone - 1) // 4)

                if blk == 0:
                    def lhs(k, jj=jj):
                        if jj < 2:
                            return wt0a[:, k, jj * 128:(jj + 1) * 128]
                        return wt0b[:, k, (jj - 2) * 128:(jj - 1) * 128]
                else:
                    def lhs(k, jj=jj, cw=cur_wt):
                        return cw[:, k, jj * 128:(jj + 1) * 128]

                ot = outp.tile([128, b], BF16, tag="ot")
                for h in range(2):
                    ps = cpsum.tile([128, 1024], F32, tag="mmps")
                    for t in (2 * h, 2 * h + 1):
                        for k in range(kc):
                            nc.tensor.matmul(
                                out=ps[:, (t % 2) * 512:(t % 2) * 512 + 512],
                                lhsT=lhs(k),
                                rhs=embt[:, k, t * 512:(t + 1) * 512],
                                start=(k == 0),
                                stop=(k == kc - 1),
                            )
                    if h == 0:
                        # ot = (ps * rn[c]) * renb[b], fused on DVE
                        nc.vector.scalar_tensor_tensor(
                            out=ot[:, :1024], in0=ps[:, :], scalar=rn[:, j:j + 1],
                            in1=renb[:, 0:1024], op0=ALU.mult, op1=ALU.mult,
                        )
                    else:
                        ot1 = scrp.tile([128, 1024], BF16, tag="ot1")
                        nc.scalar.mul(
                            out=ot1[:], in_=ps[:, :], mul=rn[:, j:j + 1]
                        )
                        nc.vector.tensor_tensor(
                            out=ot[:, 1024:], in0=ot1[:], in1=renb[:, 1024:2048],
                            op=ALU.mult,
                        )
                nc.sync.dma_start(
                    out=outv[j * 128:(j + 1) * 128, :], in_=ot[:]
                )

                if j % 6 == 3 and wsel_done < mb:
                    phase_wsel(wsel_done, egn[:, wsel_done // 4, wsel_done % 4, :])
                    wsel_done += 1
                if j == 94:
                    phi_block()

    nc.compile()
    return nc


_CACHE = {}


def _get_program():
    if "nc" not in _CACHE:
        _CACHE["nc"] = build_program()
    return _CACHE["nc"]


def make_in_maps(embeddings, labels, weight):
    emb = np.asarray(embeddings, dtype=np.float32)
    w = np.asarray(weight, dtype=np.float32)
    labels_np = np.asarray(labels).astype(np.int64)
    emb_bf = np.ascontiguousarray(emb.astype(NPBF))
    embt_bf = np.ascontiguousarray(emb_bf.T)
    w_bf = w.astype(NPBF)
    in_maps = []
    for k in range(N_CORES):
        wn = np.zeros((CSP, D), NPBF)
        wn[:CS] = w_bf[k * CS:(k + 1) * CS]
        wT = np.ascontiguousarray(wn.T)
        own = (labels_np // CS) == k
        col = labels_np - k * CS
        goff = np.where(own, col, OOB).astype(np.int64)
        goff_arr = np.ascontiguousarray(
            goff.reshape(B // 128, 128).T.astype(np.int32)
        )
        in_maps.append(
            {"emb": emb_bf, "embt": embt_bf, "wt": wT, "wn": wn,
             "goff": goff_arr}
        )
    return in_maps


def _gather(results, labels):
    labels_np = np.asarray(labels).astype(np.int64)
    bidx = np.arange(B)
    fullT = np.empty((C, B), np.float32)
    for k in range(N_CORES):
        shard = np.asarray(results[k]["out"]).reshape(CSP, B)
        fullT[k * CS:(k + 1) * CS] = shard[:CS]
        # place the device-computed 64*phi values at the target positions
        tv = np.asarray(results[k]["tv"])  # [128, mb]
        own = (labels_np // CS) == k
        ob = bidx[own]
        fullT[labels_np[ob], ob] = tv[ob % 128, ob // 128]
    return fullT.T


def kernel(embeddings, labels, weight):
    nc = _get_program()
    in_maps = make_in_maps(embeddings, labels, weight)
    res = run_bass_kernel_spmd(nc, in_maps, core_ids=list(range(N_CORES)))
    return _gather(res.results, labels)


def kernel_profiled(embeddings, labels, weight, **kw):
    """Like kernel() but also returns the BassKernelResults (exec_time_ns)."""
    nc = _get_program()
    in_maps = make_in_maps(embeddings, labels, weight)
    res = run_bass_kernel_spmd(
        nc, in_maps, core_ids=list(range(N_CORES)), trace=True, **kw
    )
    return _gather(res.results, labels), res


# revision 29
# speedup vs baseline: 1.2443x; 1.2443x over previous
"""ArcMargin head (ArcFace) distributed over 8 TRN2 NeuronCores.

Strategy (classification / tensor parallel), v7 — zero-stall ramp:
  - weight [C, D] sharded along C (12500 classes/core, padded to 12544);
    embeddings + labels replicated.  Both weight and embeddings are uploaded
    bf16 twice (natural + transposed), so the TensorEngine does no layout
    work: lhsT = wT chunk (classes stationary), rhs = raw embT.
  - TRANSPOSED logits out[c, b] = 64 * (w_c . e_hat_b): classes sit on PSUM
    partitions, so both norms fold into PSUM evacuation: 1/||w_c|| is a
    per-partition scalar, 64/||e_b|| is the per-column tensor renb [128, B].
  - renb build (no DRAM round trips - their completion latency is ~20us):
    row norms (squares on the otherwise idle Pool engine, one fused DVE
    reduce) -> erec [128,16] -> PE transpose -> rect [16,128] -> 16 tiny
    matmuls against a host-uploaded 0/1 SELECTOR pattern broadcast rect
    across all 128 partitions.  The first 10 chunks evacuate into SBUF
    staging with only the rn scale (no renb dependency), and are finalized
    once renb exists - the TensorEngine never waits for the norm chain.
  - Output is bf16 (halves the dominant HBM write traffic; rel-err budget
    2e-2 >> bf16 noise).
  - ArcFace margin: target cosines via an indirect row gather of
    weight[labels] plus a fused multiply-accumulate dot; the 2048 phi values
    leave in a tiny [128, 16] tensor and are placed into the full output
    during the host unshard (all math on device; host only does indexing).
"""

import math
import sys

import numpy as np
import ml_dtypes

for _p in ("/opt/trn_rl_repo",):
    if _p not in sys.path:
        sys.path.append(_p)

import concourse.bass as bass
import concourse.tile as tile
from concourse import bacc
from concourse import mybir
from concourse.bass_utils import run_bass_kernel_spmd

SCALE = 64.0
MARGIN = 0.5
COS_M = math.cos(MARGIN)
SIN_M = math.sin(MARGIN)
TH = math.cos(math.pi - MARGIN)
MM = math.sin(math.pi - MARGIN) * MARGIN

B, D, C = 2048, 512, 100000
N_CORES = 8
CS = C // N_CORES          # 12500 real classes per core
CSP = 12544                # padded classes per core (98 * 128)
NJ = CSP // 128            # 98 class chunks
CB = 1792                  # weight-block width (7 blocks x 14 chunks)
NBLK = CSP // CB           # 7
JPB = CB // 128            # 14 chunks per block
OOB = 1 << 30              # gather offset sentinel for "not my row"
JD = 10                    # chunks evacuated to SBUF staging (pre-renb)

NPBF = ml_dtypes.bfloat16

F32 = mybir.dt.float32
BF16 = mybir.dt.bfloat16
I32 = mybir.dt.int32
AF = mybir.ActivationFunctionType
ALU = mybir.AluOpType


def build_program(b=B, d=D, csp=CSP):
    """Build the (SPMD-uniform) single-core Bass program."""
    mb = b // 128          # 16 batch row-chunks
    kc = d // 128          # 4 contraction chunks
    nc = bacc.Bacc()

    emb_d = nc.declare_dram_parameter("emb", [b, d], BF16, isOutput=False)
    embt_d = nc.declare_dram_parameter("embt", [d, b], BF16, isOutput=False)
    wt_d = nc.declare_dram_parameter("wt", [d, csp], BF16, isOutput=False)
    wn_d = nc.declare_dram_parameter("wn", [csp, d], BF16, isOutput=False)
    goff_d = nc.declare_dram_parameter("goff", [128, mb], I32, isOutput=False)
    ident_d = nc.declare_dram_parameter("ident", [128, 128], F32, isOutput=False)
    dsel_d = nc.declare_dram_parameter("dsel", [16, b], BF16, isOutput=False)
    # flat transposed output [c * B + b]
    out_d = nc.declare_dram_parameter("out", [csp * b, 1], BF16, isOutput=True)
    tv_d = nc.declare_dram_parameter("tv", [128, mb], F32, isOutput=True)

    with tile.TileContext(nc) as tc:
        with (
            tc.tile_pool(name="const", bufs=1) as constp,
            tc.tile_pool(name="persist", bufs=1) as persist,
            tc.tile_pool(name="eld", bufs=1) as eldp,
            tc.tile_pool(name="wtp", bufs=2) as wtp,
            tc.tile_pool(name="wnp", bufs=3) as wnp,
            tc.tile_pool(name="scr", bufs=2) as scrp,
            tc.tile_pool(name="smp", bufs=4) as smp,
            tc.tile_pool(name="outp", bufs=3) as outp,
            tc.tile_pool(name="stg", bufs=1) as stgp,
            tc.tile_pool(name="cpsum", bufs=4, space="PSUM") as cpsum,
        ):
            zb = constp.tile([128, 1], F32, tag="zb")
            nc.vector.memset(zb[:], 0.0)
            epsb = constp.tile([128, 1], F32, tag="epsb")
            nc.vector.memset(epsb[:], 1e-24)
            s2b = constp.tile([128, 1], F32, tag="s2b")
            nc.vector.memset(s2b[:], SCALE * SCALE)
            identf = constp.tile([128, 128], F32, tag="identf")
            dsel = constp.tile([16, b], BF16, tag="dsel")

            embt = persist.tile([128, kc, b], BF16)     # e^T raw (matmul rhs)
            renb = persist.tile([128, b], BF16)         # 64/||e_b|| bcast
            sqg = persist.tile([128, mb // 2, d], BF16)  # e^2 scratch (gpsimd)
            essq = persist.tile([128, mb], F32)         # per-row sum(e^2)
            erec = persist.tile([128, mb], F32)         # 64/||e_b||, natural
            rect = persist.tile([16, 128], BF16)        # erec transposed
            nsq = persist.tile([128, NJ], F32)          # per-class sum(w^2)
            nrm = persist.tile([128, NJ], F32)
            rn = persist.tile([128, NJ], F32)           # 1/||w_c||
            svec = persist.tile([128, mb], F32)         # 64*cos(target)
            tval = persist.tile([128, mb], F32)         # 64*phi / else-branch
            gofft = persist.tile([128, mb], I32)
            stg = stgp.tile([128, JD, b], BF16)         # staged rn-scaled out

            outv = out_d[:].rearrange("(c b) o -> c (b o)", b=b)  # [csp, b]

            # ---------------- DMA helpers ----------------
            def wt_blk(blk):
                t = wtp.tile([128, kc, CB], BF16, tag="wt", name=f"wt_{blk}")
                nc.sync.dma_start(
                    out=t[:],
                    in_=wt_d[:, blk * CB:(blk + 1) * CB].rearrange(
                        "(k p) c -> p k c", p=128
                    ),
                )
                return t

            wn_tiles = {}

            def wn_g(g):
                r0 = g * 512
                ng = min(4, NJ - g * 4)
                t = wnp.tile([128, 4, d], BF16, tag="wn", name=f"wn_{g}")
                nc.sync.dma_start(
                    out=t[:, :ng, :],
                    in_=wn_d[r0:r0 + ng * 128, :].rearrange(
                        "(g2 p) dd -> p g2 dd", p=128
                    ),
                )
                wn_tiles[g] = t

            # ---------------- compute helpers ----------------
            def wnorm_chunk(c):
                sq = scrp.tile([128, d], BF16, tag="sqw")
                nc.scalar.activation(
                    out=sq[:], in_=wn_tiles[c // 4][:, c % 4, :], func=AF.Square,
                    bias=zb[:], accum_out=nsq[:, c:c + 1],
                )

            def rn_fin(g):
                s0 = g * 4
                s1 = min(s0 + 4, NJ)
                nc.scalar.activation(
                    out=nrm[:, s0:s1], in_=nsq[:, s0:s1], func=AF.Sqrt, bias=epsb[:]
                )
                nc.vector.reciprocal(out=rn[:, s0:s1], in_=nrm[:, s0:s1])

            def phase_wsel(m):
                # gather this chunk's owned target weight rows from the shard
                wsld = scrp.tile([128, d], BF16, tag="wsld", name=f"ws_{m}")
                nc.gpsimd.indirect_dma_start(
                    out=wsld[:],
                    out_offset=None,
                    in_=wn_d[:],
                    in_offset=bass.IndirectOffsetOnAxis(
                        ap=gofft[:, m:m + 1], axis=0
                    ),
                    bounds_check=csp - 1,
                    oob_is_err=False,
                )
                sq = scrp.tile([128, d], BF16, tag="sqs")
                ssq = smp.tile([128, 1], F32, tag="ssqs")
                nc.scalar.activation(
                    out=sq[:], in_=wsld[:], func=AF.Square, bias=zb[:],
                    accum_out=ssq[:],
                )
                nrm_s = smp.tile([128, 1], F32, tag="nrms")
                nc.scalar.activation(out=nrm_s[:], in_=ssq[:], func=AF.Sqrt, bias=epsb[:])
                rec = smp.tile([128, 1], F32, tag="recs")
                nc.vector.reciprocal(out=rec[:], in_=nrm_s[:])
                # raw dot sum(wsld * e_raw); scale by 1/||w|| and 64/||e|| after
                ttr = scrp.tile([128, d], BF16, tag="ttr")
                dotm = smp.tile([128, 1], F32, tag="dotm")
                nc.vector.scalar_tensor_tensor(
                    out=ttr[:], in0=wsld[:], scalar=1.0,
                    in1=egn[:, m // 4, m % 4, :],
                    op0=ALU.mult, op1=ALU.mult, accum_out=dotm[:],
                )
                dw = smp.tile([128, 1], F32, tag="dw")
                nc.vector.tensor_tensor(
                    out=dw[:], in0=dotm[:], in1=rec[:], op=ALU.mult
                )
                nc.vector.tensor_tensor(
                    out=svec[:, m:m + 1], in0=dw[:], in1=erec[:, m:m + 1],
                    op=ALU.mult,
                )

            def phi_block():
                s2 = smp.tile([128, mb], F32, tag="s2")
                nc.scalar.activation(
                    out=s2[:], in_=svec[:], func=AF.Square, bias=zb[:]
                )
                rl = smp.tile([128, mb], F32, tag="rl")
                nc.scalar.activation(
                    out=rl[:], in_=s2[:], func=AF.Relu, bias=s2b[:], scale=-1.0
                )
                sn = smp.tile([128, mb], F32, tag="sn")
                nc.scalar.activation(out=sn[:], in_=rl[:], func=AF.Sqrt, bias=zb[:])
                pc = smp.tile([128, mb], F32, tag="pc")
                nc.vector.tensor_scalar_mul(out=pc[:], in0=svec[:], scalar1=COS_M)
                smt = smp.tile([128, mb], F32, tag="smt")
                nc.vector.tensor_scalar_mul(out=smt[:], in0=sn[:], scalar1=SIN_M)
                ph = smp.tile([128, mb], F32, tag="ph")
                nc.vector.tensor_tensor(
                    out=ph[:], in0=pc[:], in1=smt[:], op=ALU.subtract
                )
                eb = smp.tile([128, mb], F32, tag="eb")
                nc.vector.tensor_scalar_add(
                    out=eb[:], in0=svec[:], scalar1=-SCALE * MM
                )
                mk = smp.tile([128, mb], mybir.dt.uint8, tag="mk")
                nc.vector.tensor_scalar(
                    out=mk[:], in0=svec[:], scalar1=SCALE * TH, scalar2=None,
                    op0=ALU.is_gt,
                )
                nc.vector.select(out=tval[:], mask=mk[:], on_true=ph[:], on_false=eb[:])
                nc.sync.dma_start(out=tv_d[:], in_=tval[:])

            # ---------------- prologue (DMA order = ring order) ----------------
            for hh in range(2):
                nc.sync.dma_start(
                    out=embt[:, :, hh * 1024:(hh + 1) * 1024],
                    in_=embt_d[:, hh * 1024:(hh + 1) * 1024].rearrange(
                        "(k p) c -> p k c", p=128
                    ),
                )
            wt0a = wtp.tile([128, kc, 256], BF16, tag="wt0a")
            nc.sync.dma_start(
                out=wt0a[:],
                in_=wt_d[:, 0:256].rearrange("(k p) c -> p k c", p=128),
            )
            wn_g(0)
            wt0b = wtp.tile([128, kc, CB - 256], BF16, tag="wt0b")
            nc.sync.dma_start(
                out=wt0b[:],
                in_=wt_d[:, 256:CB].rearrange("(k p) c -> p k c", p=128),
            )
            wn_g(1)
            egn = eldp.tile([128, mb // 4, 4, d], BF16, tag="egn")
            for g in range(4):
                nc.sync.dma_start(
                    out=egn[:, g, :, :],
                    in_=emb_d[g * 512:(g + 1) * 512, :].rearrange(
                        "(g2 p) dd -> p g2 dd", p=128
                    ),
                )
            nc.sync.dma_start(out=gofft[:], in_=goff_d[:])
            nc.sync.dma_start(out=identf[:], in_=ident_d[:])
            nc.sync.dma_start(out=dsel[:], in_=dsel_d[:])

            # e^2 on the Pool engine (it is idle; squares wait on the egn DMAs
            # without blocking the Scalar/Vector evacuation streams); two
            # rounds of 8 through one half-size scratch
            for m in range(mb // 2):
                nc.gpsimd.tensor_tensor(
                    out=sqg[:, m, :], in0=egn[:, m // 4, m % 4, :],
                    in1=egn[:, m // 4, m % 4, :], op=ALU.mult,
                )

            wdone = 0
            while wdone < 8:
                wnorm_chunk(wdone)
                wdone += 1
                if wdone % 4 == 0:
                    rn_fin(wdone // 4 - 1)

            # ---------------- main loop over class chunks ----------------
            cur_wt = None
            nxt_wt = None
            wsel_done = 0
            dsent = 0
            for j in range(NJ):
                blk, jj = divmod(j, JPB)
                if jj == 0:
                    if blk > 0:
                        cur_wt = nxt_wt
                    if blk < NBLK - 1:
                        nxt_wt = wt_blk(blk + 1)
                if j % 4 == 0:
                    g = j // 4 + 2
                    if g * 4 < NJ:
                        wn_g(g)
                while wdone < min(NJ, j + 9):
                    wnorm_chunk(wdone)
                    wdone += 1
                    if wdone % 4 == 0 or wdone == NJ:
                        rn_fin((wdone - 1) // 4)

                if j == 4:
                    nc.vector.tensor_reduce(
                        out=essq[:, 0:mb // 2], in_=sqg[:],
                        axis=mybir.AxisListType.X, op=ALU.add,
                    )
                    for m in range(mb // 2, mb):
                        nc.gpsimd.tensor_tensor(
                            out=sqg[:, m - mb // 2, :],
                            in0=egn[:, m // 4, m % 4, :],
                            in1=egn[:, m // 4, m % 4, :], op=ALU.mult,
                        )
                if j == 5:
                    # essq = per-row sum of e^2 (one fused reduce), then
                    # erec = 64/||e|| = 1/sqrt(essq/4096 + eps)
                    nc.vector.tensor_reduce(
                        out=essq[:, mb // 2:mb], in_=sqg[:],
                        axis=mybir.AxisListType.X, op=ALU.add,
                    )
                    enrm = smp.tile([128, mb], F32, tag="enrm")
                    nc.scalar.activation(
                        out=enrm[:], in_=essq[:], func=AF.Sqrt, bias=epsb[:],
                        scale=1.0 / (SCALE * SCALE),
                    )
                    nc.vector.reciprocal(out=erec[:], in_=enrm[:])

                if j == 8:
                    # renb[p, x] = erec[x%128, x//128] for every partition p:
                    # PE transpose of erec, then 16 selector matmuls
                    pst = cpsum.tile([128, 1024], F32, tag="mmps", name="ps_tr")
                    nc.tensor.transpose(
                        out=pst[0:mb, 0:128], in_=erec[:], identity=identf[:]
                    )
                    nc.vector.tensor_copy(out=rect[:], in_=pst[0:mb, 0:128])
                    for hh in range(2):
                        psr = cpsum.tile(
                            [128, 1024], F32, tag="mmps", name=f"ps_re{hh}"
                        )
                        for q2 in range(8):
                            q = hh * 8 + q2
                            nc.tensor.matmul(
                                out=psr[:, q2 * 128:(q2 + 1) * 128],
                                lhsT=dsel[:, q * 128:(q + 1) * 128],
                                rhs=rect[:],
                                start=True, stop=True,
                            )
                        if hh == 0:
                            nc.vector.tensor_copy(out=renb[:, 0:1024], in_=psr[:])
                        else:
                            nc.scalar.copy(out=renb[:, 1024:2048], in_=psr[:])

                if blk == 0:
                    def lhs(k, jj=jj):
                        if jj < 2:
                            return wt0a[:, k, jj * 128:(jj + 1) * 128]
                        return wt0b[:, k, (jj - 2) * 128:(jj - 1) * 128]
                else:
                    def lhs(k, jj=jj, cw=cur_wt):
                        return cw[:, k, jj * 128:(jj + 1) * 128]

                staged = j < JD
                ot = None if staged else outp.tile([128, b], BF16, tag="ot")
                for h in range(2):
                    ps = cpsum.tile([128, 1024], F32, tag="mmps")
                    for t in (2 * h, 2 * h + 1):
                        for k in range(kc):
                            nc.tensor.matmul(
                                out=ps[:, (t % 2) * 512:(t % 2) * 512 + 512],
                                lhsT=lhs(k),
                                rhs=embt[:, k, t * 512:(t + 1) * 512],
                                start=(k == 0),
                                stop=(k == kc - 1),
                            )
                    if staged:
                        # rn-only evacuation into SBUF staging (no renb dep)
                        if h == 0:
                            nc.vector.tensor_scalar_mul(
                                out=stg[:, j, 0:1024], in0=ps[:, :],
                                scalar1=rn[:, j:j + 1],
                            )
                        else:
                            nc.scalar.mul(
                                out=stg[:, j, 1024:2048], in_=ps[:, :],
                                mul=rn[:, j:j + 1],
                            )
                    elif h == 0:
                        # ot = (ps * rn[c]) * renb[b], fused on DVE
                        nc.vector.scalar_tensor_tensor(
                            out=ot[:, :1024], in0=ps[:, :], scalar=rn[:, j:j + 1],
                            in1=renb[:, 0:1024], op0=ALU.mult, op1=ALU.mult,
                        )
                    else:
                        ot1 = scrp.tile([128, 1024], BF16, tag="ot1")
                        nc.scalar.mul(
                            out=ot1[:], in_=ps[:, :], mul=rn[:, j:j + 1]
                        )
                        nc.vector.tensor_tensor(
                            out=ot[:, 1024:], in0=ot1[:], in1=renb[:, 1024:2048],
                            op=ALU.mult,
                        )
                if not staged:
                    nc.sync.dma_start(
                        out=outv[j * 128:(j + 1) * 128, :], in_=ot[:]
                    )

                # finalize one staged chunk per j once renb exists
                if j >= 11 and dsent < JD:
                    jd = dsent
                    otd = outp.tile([128, b], BF16, tag="ot")
                    nc.vector.tensor_tensor(
                        out=otd[:, 0:1024], in0=stg[:, jd, 0:1024],
                        in1=renb[:, 0:1024], op=ALU.mult,
                    )
                    nc.vector.tensor_tensor(
                        out=otd[:, 1024:2048], in0=stg[:, jd, 1024:2048],
                        in1=renb[:, 1024:2048], op=ALU.mult,
                    )
                    nc.sync.dma_start(
                        out=outv[jd * 128:(jd + 1) * 128, :], in_=otd[:]
                    )
                    dsent += 1

                if j >= 9 and (j - 9) % 5 == 0 and wsel_done < mb:
                    phase_wsel(wsel_done)
                    wsel_done += 1
                if j == 88:
                    phi_block()

    nc.compile()
    return nc


_CACHE = {}


def _get_program():
    if "nc" not in _CACHE:
        _CACHE["nc"] = build_program()
    return _CACHE["nc"]


def make_in_maps(embeddings, labels, weight):
    emb = np.asarray(embeddings, dtype=np.float32)
    w = np.asarray(weight, dtype=np.float32)
    labels_np = np.asarray(labels).astype(np.int64)
    emb_bf = np.ascontiguousarray(emb.astype(NPBF))
    embt_bf = np.ascontiguousarray(emb_bf.T)
    w_bf = w.astype(NPBF)
    ident = np.eye(128, dtype=np.float32)
    dsel = np.zeros((16, B), NPBF)
    for k in range(16):
        dsel[k, k * 128:(k + 1) * 128] = 1.0
    in_maps = []
    for k in range(N_CORES):
        wn = np.zeros((CSP, D), NPBF)
        wn[:CS] = w_bf[k * CS:(k + 1) * CS]
        wT = np.ascontiguousarray(wn.T)
        own = (labels_np // CS) == k
        col = labels_np - k * CS
        goff = np.where(own, col, OOB).astype(np.int64)
        goff_arr = np.ascontiguousarray(
            goff.reshape(B // 128, 128).T.astype(np.int32)
        )
        in_maps.append(
            {"emb": emb_bf, "embt": embt_bf, "wt": wT, "wn": wn,
             "goff": goff_arr, "ident": ident, "dsel": dsel}
        )
    return in_maps


def _gather(results, labels):
    labels_np = np.asarray(labels).astype(np.int64)
    bidx = np.arange(B)
    fullT = np.empty((C, B), np.float32)
    for k in range(N_CORES):
        shard = np.asarray(results[k]["out"]).reshape(CSP, B)
        fullT[k * CS:(k + 1) * CS] = shard[:CS]
        # place the device-computed 64*phi values at the target positions
        tv = np.asarray(results[k]["tv"])  # [128, mb]
        own = (labels_np // CS) == k
        ob = bidx[own]
        fullT[labels_np[ob], ob] = tv[ob % 128, ob // 128]
    return fullT.T


def kernel(embeddings, labels, weight):
    nc = _get_program()
    in_maps = make_in_maps(embeddings, labels, weight)
    res = run_bass_kernel_spmd(nc, in_maps, core_ids=list(range(N_CORES)))
    return _gather(res.results, labels)


def kernel_profiled(embeddings, labels, weight, **kw):
    """Like kernel() but also returns the BassKernelResults (exec_time_ns)."""
    nc = _get_program()
    in_maps = make_in_maps(embeddings, labels, weight)
    res = run_bass_kernel_spmd(
        nc, in_maps, core_ids=list(range(N_CORES)), trace=True, **kw
    )
    return _gather(res.results, labels), res


# revision 30
# speedup vs baseline: 1.2538x; 1.0076x over previous
"""ArcMargin head (ArcFace) distributed over 8 TRN2 NeuronCores.

Strategy (classification / tensor parallel), v7 — zero-stall ramp:
  - weight [C, D] sharded along C (12500 classes/core, padded to 12544);
    embeddings + labels replicated.  Both weight and embeddings are uploaded
    bf16 twice (natural + transposed), so the TensorEngine does no layout
    work: lhsT = wT chunk (classes stationary), rhs = raw embT.
  - TRANSPOSED logits out[c, b] = 64 * (w_c . e_hat_b): classes sit on PSUM
    partitions, so both norms fold into PSUM evacuation: 1/||w_c|| is a
    per-partition scalar, 64/||e_b|| is the per-column tensor renb [128, B].
  - renb build (no DRAM round trips - their completion latency is ~20us):
    row norms (squares on the otherwise idle Pool engine, one fused DVE
    reduce) -> erec [128,16] -> PE transpose -> rect [16,128] -> 16 tiny
    matmuls against a host-uploaded 0/1 SELECTOR pattern broadcast rect
    across all 128 partitions.  The first 10 chunks evacuate into SBUF
    staging with only the rn scale (no renb dependency), and are finalized
    once renb exists - the TensorEngine never waits for the norm chain.
  - Output is bf16 (halves the dominant HBM write traffic; rel-err budget
    2e-2 >> bf16 noise).
  - ArcFace margin: target cosines via an indirect row gather of
    weight[labels] plus a fused multiply-accumulate dot; the 2048 phi values
    leave in a tiny [128, 16] tensor and are placed into the full output
    during the host unshard (all math on device; host only does indexing).
"""

import math
import sys

import numpy as np
import ml_dtypes

for _p in ("/opt/trn_rl_repo",):
    if _p not in sys.path:
        sys.path.append(_p)

import concourse.bass as bass
import concourse.tile as tile
from concourse import bacc
from concourse import mybir
from concourse.bass_utils import run_bass_kernel_spmd

SCALE = 64.0
MARGIN = 0.5
COS_M = math.cos(MARGIN)
SIN_M = math.sin(MARGIN)
TH = math.cos(math.pi - MARGIN)
MM = math.sin(math.pi - MARGIN) * MARGIN

B, D, C = 2048, 512, 100000
N_CORES = 8
CS = C // N_CORES          # 12500 real classes per core
CSP = 12544                # padded classes per core (98 * 128)
NJ = CSP // 128            # 98 class chunks
CB = 1792                  # weight-block width (7 blocks x 14 chunks)
NBLK = CSP // CB           # 7
JPB = CB // 128            # 14 chunks per block
OOB = 1 << 30              # gather offset sentinel for "not my row"
JD = 10                    # chunks evacuated to SBUF staging (pre-renb)

NPBF = ml_dtypes.bfloat16

F32 = mybir.dt.float32
BF16 = mybir.dt.bfloat16
I32 = mybir.dt.int32
AF = mybir.ActivationFunctionType
ALU = mybir.AluOpType


def build_program(b=B, d=D, csp=CSP):
    """Build the (SPMD-uniform) single-core Bass program."""
    mb = b // 128          # 16 batch row-chunks
    kc = d // 128          # 4 contraction chunks
    nc = bacc.Bacc()

    emb_d = nc.declare_dram_parameter("emb", [b, d], BF16, isOutput=False)
    embt_d = nc.declare_dram_parameter("embt", [d, b], BF16, isOutput=False)
    wt_d = nc.declare_dram_parameter("wt", [d, csp], BF16, isOutput=False)
    wn_d = nc.declare_dram_parameter("wn", [csp, d], BF16, isOutput=False)
    goff_d = nc.declare_dram_parameter("goff", [128, mb], I32, isOutput=False)
    ident_d = nc.declare_dram_parameter("ident", [128, 128], F32, isOutput=False)
    dsel_d = nc.declare_dram_parameter("dsel", [16, b], BF16, isOutput=False)
    # flat transposed output [c * B + b]
    out_d = nc.declare_dram_parameter("out", [csp * b, 1], BF16, isOutput=True)
    tv_d = nc.declare_dram_parameter("tv", [128, mb], F32, isOutput=True)

    with tile.TileContext(nc) as tc:
        with (
            tc.tile_pool(name="const", bufs=1) as constp,
            tc.tile_pool(name="persist", bufs=1) as persist,
            tc.tile_pool(name="eld", bufs=1) as eldp,
            tc.tile_pool(name="wtp", bufs=2) as wtp,
            tc.tile_pool(name="wnp", bufs=3) as wnp,
            tc.tile_pool(name="scr", bufs=2) as scrp,
            tc.tile_pool(name="smp", bufs=4) as smp,
            tc.tile_pool(name="outp", bufs=3) as outp,
            tc.tile_pool(name="stg", bufs=1) as stgp,
            tc.tile_pool(name="cpsum", bufs=4, space="PSUM") as cpsum,
        ):
            zb = constp.tile([128, 1], F32, tag="zb")
            nc.vector.memset(zb[:], 0.0)
            epsb = constp.tile([128, 1], F32, tag="epsb")
            nc.vector.memset(epsb[:], 1e-24)
            s2b = constp.tile([128, 1], F32, tag="s2b")
            nc.vector.memset(s2b[:], SCALE * SCALE)
            identf = constp.tile([128, 128], F32, tag="identf")
            dsel = constp.tile([16, b], BF16, tag="dsel")

            embt = persist.tile([128, kc, b], BF16)     # e^T raw (matmul rhs)
            renb = persist.tile([128, b], BF16)         # 64/||e_b|| bcast
            sqg = persist.tile([128, mb // 2, d], BF16)  # e^2 scratch (gpsimd)
            essq = persist.tile([128, mb], F32)         # per-row sum(e^2)
            erec = persist.tile([128, mb], F32)         # 64/||e_b||, natural
            rect = persist.tile([16, 128], BF16)        # erec transposed
            nsq = persist.tile([128, NJ], F32)          # per-class sum(w^2)
            nrm = persist.tile([128, NJ], F32)
            rn = persist.tile([128, NJ], F32)           # 1/||w_c||
            svec = persist.tile([128, mb], F32)         # 64*cos(target)
            tval = persist.tile([128, mb], F32)         # 64*phi / else-branch
            gofft = persist.tile([128, mb], I32)
            stg = stgp.tile([128, JD, b], BF16)         # staged rn-scaled out

            outv = out_d[:].rearrange("(c b) o -> c (b o)", b=b)  # [csp, b]

            # ---------------- DMA helpers ----------------
            def wt_blk(blk):
                t = wtp.tile([128, kc, CB], BF16, tag="wt", name=f"wt_{blk}")
                nc.sync.dma_start(
                    out=t[:],
                    in_=wt_d[:, blk * CB:(blk + 1) * CB].rearrange(
                        "(k p) c -> p k c", p=128
                    ),
                )
                return t

            wn_tiles = {}

            def wn_g(g):
                r0 = g * 512
                ng = min(4, NJ - g * 4)
                t = wnp.tile([128, 4, d], BF16, tag="wn", name=f"wn_{g}")
                nc.sync.dma_start(
                    out=t[:, :ng, :],
                    in_=wn_d[r0:r0 + ng * 128, :].rearrange(
                        "(g2 p) dd -> p g2 dd", p=128
                    ),
                )
                wn_tiles[g] = t

            # ---------------- compute helpers ----------------
            def wnorm_chunk(c):
                sq = scrp.tile([128, d], BF16, tag="sqw")
                nc.scalar.activation(
                    out=sq[:], in_=wn_tiles[c // 4][:, c % 4, :], func=AF.Square,
                    bias=zb[:], accum_out=nsq[:, c:c + 1],
                )

            def rn_fin(g):
                s0 = g * 4
                s1 = min(s0 + 4, NJ)
                nc.scalar.activation(
                    out=nrm[:, s0:s1], in_=nsq[:, s0:s1], func=AF.Sqrt, bias=epsb[:]
                )
                nc.vector.reciprocal(out=rn[:, s0:s1], in_=nrm[:, s0:s1])

            def phase_wsel(m):
                # gather this chunk's owned target weight rows from the shard
                wsld = scrp.tile([128, d], BF16, tag="wsld", name=f"ws_{m}")
                nc.gpsimd.indirect_dma_start(
                    out=wsld[:],
                    out_offset=None,
                    in_=wn_d[:],
                    in_offset=bass.IndirectOffsetOnAxis(
                        ap=gofft[:, m:m + 1], axis=0
                    ),
                    bounds_check=csp - 1,
                    oob_is_err=False,
                )
                sq = scrp.tile([128, d], BF16, tag="sqs")
                ssq = smp.tile([128, 1], F32, tag="ssqs")
                nc.scalar.activation(
                    out=sq[:], in_=wsld[:], func=AF.Square, bias=zb[:],
                    accum_out=ssq[:],
                )
                nrm_s = smp.tile([128, 1], F32, tag="nrms")
                nc.scalar.activation(out=nrm_s[:], in_=ssq[:], func=AF.Sqrt, bias=epsb[:])
                rec = smp.tile([128, 1], F32, tag="recs")
                nc.vector.reciprocal(out=rec[:], in_=nrm_s[:])
                # raw dot sum(wsld * e_raw); scale by 1/||w|| and 64/||e|| after
                ttr = scrp.tile([128, d], BF16, tag="ttr")
                dotm = smp.tile([128, 1], F32, tag="dotm")
                nc.vector.scalar_tensor_tensor(
                    out=ttr[:], in0=wsld[:], scalar=1.0,
                    in1=egn[:, m // 4, m % 4, :],
                    op0=ALU.mult, op1=ALU.mult, accum_out=dotm[:],
                )
                dw = smp.tile([128, 1], F32, tag="dw")
                nc.vector.tensor_tensor(
                    out=dw[:], in0=dotm[:], in1=rec[:], op=ALU.mult
                )
                nc.vector.tensor_tensor(
                    out=svec[:, m:m + 1], in0=dw[:], in1=erec[:, m:m + 1],
                    op=ALU.mult,
                )

            def phi_block():
                s2 = smp.tile([128, mb], F32, tag="s2")
                nc.scalar.activation(
                    out=s2[:], in_=svec[:], func=AF.Square, bias=zb[:]
                )
                rl = smp.tile([128, mb], F32, tag="rl")
                nc.scalar.activation(
                    out=rl[:], in_=s2[:], func=AF.Relu, bias=s2b[:], scale=-1.0
                )
                sn = smp.tile([128, mb], F32, tag="sn")
                nc.scalar.activation(out=sn[:], in_=rl[:], func=AF.Sqrt, bias=zb[:])
                pc = smp.tile([128, mb], F32, tag="pc")
                nc.vector.tensor_scalar_mul(out=pc[:], in0=svec[:], scalar1=COS_M)
                smt = smp.tile([128, mb], F32, tag="smt")
                nc.vector.tensor_scalar_mul(out=smt[:], in0=sn[:], scalar1=SIN_M)
                ph = smp.tile([128, mb], F32, tag="ph")
                nc.vector.tensor_tensor(
                    out=ph[:], in0=pc[:], in1=smt[:], op=ALU.subtract
                )
                eb = smp.tile([128, mb], F32, tag="eb")
                nc.vector.tensor_scalar_add(
                    out=eb[:], in0=svec[:], scalar1=-SCALE * MM
                )
                mk = smp.tile([128, mb], mybir.dt.uint8, tag="mk")
                nc.vector.tensor_scalar(
                    out=mk[:], in0=svec[:], scalar1=SCALE * TH, scalar2=None,
                    op0=ALU.is_gt,
                )
                nc.vector.select(out=tval[:], mask=mk[:], on_true=ph[:], on_false=eb[:])
                nc.sync.dma_start(out=tv_d[:], in_=tval[:])

            # ---------------- prologue (DMA order = ring order) ----------------
            nc.sync.dma_start(
                out=embt[:, :, 0:1024],
                in_=embt_d[:, 0:1024].rearrange("(k p) c -> p k c", p=128),
            )
            wt0a = wtp.tile([128, kc, 256], BF16, tag="wt0a")
            nc.sync.dma_start(
                out=wt0a[:],
                in_=wt_d[:, 0:256].rearrange("(k p) c -> p k c", p=128),
            )
            nc.sync.dma_start(
                out=embt[:, :, 1024:2048],
                in_=embt_d[:, 1024:2048].rearrange("(k p) c -> p k c", p=128),
            )
            wn_g(0)
            wt0b = wtp.tile([128, kc, CB - 256], BF16, tag="wt0b")
            nc.sync.dma_start(
                out=wt0b[:],
                in_=wt_d[:, 256:CB].rearrange("(k p) c -> p k c", p=128),
            )
            wn_g(1)
            egn = eldp.tile([128, mb // 4, 4, d], BF16, tag="egn")
            for g in range(4):
                nc.sync.dma_start(
                    out=egn[:, g, :, :],
                    in_=emb_d[g * 512:(g + 1) * 512, :].rearrange(
                        "(g2 p) dd -> p g2 dd", p=128
                    ),
                )
            nc.sync.dma_start(out=gofft[:], in_=goff_d[:])
            nc.sync.dma_start(out=identf[:], in_=ident_d[:])
            nc.sync.dma_start(out=dsel[:], in_=dsel_d[:])

            # e^2 on the Pool engine (it is idle; squares wait on the egn DMAs
            # without blocking the Scalar/Vector evacuation streams); two
            # rounds of 8 through one half-size scratch
            for m in range(mb // 2):
                nc.gpsimd.tensor_tensor(
                    out=sqg[:, m, :], in0=egn[:, m // 4, m % 4, :],
                    in1=egn[:, m // 4, m % 4, :], op=ALU.mult,
                )

            wdone = 0
            while wdone < 8:
                wnorm_chunk(wdone)
                wdone += 1
                if wdone % 4 == 0:
                    rn_fin(wdone // 4 - 1)

            # ---------------- main loop over class chunks ----------------
            cur_wt = None
            nxt_wt = None
            wsel_done = 0
            dsent = 0
            for j in range(NJ):
                blk, jj = divmod(j, JPB)
                if jj == 0:
                    if blk > 0:
                        cur_wt = nxt_wt
                    if blk < NBLK - 1:
                        nxt_wt = wt_blk(blk + 1)
                if j % 4 == 0:
                    g = j // 4 + 2
                    if g * 4 < NJ:
                        wn_g(g)
                while wdone < min(NJ, j + 9):
                    wnorm_chunk(wdone)
                    wdone += 1
                    if wdone % 4 == 0 or wdone == NJ:
                        rn_fin((wdone - 1) // 4)

                if j == 4:
                    nc.vector.tensor_reduce(
                        out=essq[:, 0:mb // 2], in_=sqg[:],
                        axis=mybir.AxisListType.X, op=ALU.add,
                    )
                    for m in range(mb // 2, mb):
                        nc.gpsimd.tensor_tensor(
                            out=sqg[:, m - mb // 2, :],
                            in0=egn[:, m // 4, m % 4, :],
                            in1=egn[:, m // 4, m % 4, :], op=ALU.mult,
                        )
                if j == 5:
                    # essq = per-row sum of e^2 (one fused reduce), then
                    # erec = 64/||e|| = 1/sqrt(essq/4096 + eps)
                    nc.vector.tensor_reduce(
                        out=essq[:, mb // 2:mb], in_=sqg[:],
                        axis=mybir.AxisListType.X, op=ALU.add,
                    )
                    enrm = smp.tile([128, mb], F32, tag="enrm")
                    nc.scalar.activation(
                        out=enrm[:], in_=essq[:], func=AF.Sqrt, bias=epsb[:],
                        scale=1.0 / (SCALE * SCALE),
                    )
                    nc.vector.reciprocal(out=erec[:], in_=enrm[:])

                if j == 8:
                    # renb[p, x] = erec[x%128, x//128] for every partition p:
                    # PE transpose of erec, then 16 selector matmuls
                    pst = cpsum.tile([128, 1024], F32, tag="mmps", name="ps_tr")
                    nc.tensor.transpose(
                        out=pst[0:mb, 0:128], in_=erec[:], identity=identf[:]
                    )
                    nc.vector.tensor_copy(out=rect[:], in_=pst[0:mb, 0:128])
                    for hh in range(2):
                        psr = cpsum.tile(
                            [128, 1024], F32, tag="mmps", name=f"ps_re{hh}"
                        )
                        for q2 in range(8):
                            q = hh * 8 + q2
                            nc.tensor.matmul(
                                out=psr[:, q2 * 128:(q2 + 1) * 128],
                                lhsT=dsel[:, q * 128:(q + 1) * 128],
                                rhs=rect[:],
                                start=True, stop=True,
                            )
                        if hh == 0:
                            nc.vector.tensor_copy(out=renb[:, 0:1024], in_=psr[:])
                        else:
                            nc.scalar.copy(out=renb[:, 1024:2048], in_=psr[:])

                if blk == 0:
                    def lhs(k, jj=jj):
                        if jj < 2:
                            return wt0a[:, k, jj * 128:(jj + 1) * 128]
                        return wt0b[:, k, (jj - 2) * 128:(jj - 1) * 128]
                else:
                    def lhs(k, jj=jj, cw=cur_wt):
                        return cw[:, k, jj * 128:(jj + 1) * 128]

                staged = j < JD
                ot = None if staged else outp.tile([128, b], BF16, tag="ot")
                for h in range(2):
                    ps = cpsum.tile([128, 1024], F32, tag="mmps")
                    for t in (2 * h, 2 * h + 1):
                        for k in range(kc):
                            nc.tensor.matmul(
                                out=ps[:, (t % 2) * 512:(t % 2) * 512 + 512],
                                lhsT=lhs(k),
                                rhs=embt[:, k, t * 512:(t + 1) * 512],
                                start=(k == 0),
                                stop=(k == kc - 1),
                            )
                    if staged:
                        # rn-only evacuation into SBUF staging (no renb dep)
                        if h == 0:
                            nc.vector.tensor_scalar_mul(
                                out=stg[:, j, 0:1024], in0=ps[:, :],
                                scalar1=rn[:, j:j + 1],
                            )
                        else:
                            nc.scalar.mul(
                                out=stg[:, j, 1024:2048], in_=ps[:, :],
                                mul=rn[:, j:j + 1],
                            )
                    elif h == 0:
                        # ot = (ps * rn[c]) * renb[b], fused on DVE
                        nc.vector.scalar_tensor_tensor(
                            out=ot[:, :1024], in0=ps[:, :], scalar=rn[:, j:j + 1],
                            in1=renb[:, 0:1024], op0=ALU.mult, op1=ALU.mult,
                        )
                    else:
                        ot1 = scrp.tile([128, 1024], BF16, tag="ot1")
                        nc.scalar.mul(
                            out=ot1[:], in_=ps[:, :], mul=rn[:, j:j + 1]
                        )
                        nc.vector.tensor_tensor(
                            out=ot[:, 1024:], in0=ot1[:], in1=renb[:, 1024:2048],
                            op=ALU.mult,
                        )
                if not staged:
                    nc.sync.dma_start(
                        out=outv[j * 128:(j + 1) * 128, :], in_=ot[:]
                    )

                # finalize one staged chunk every other j once renb exists
                if j >= 11 and j % 2 == 1 and dsent < JD:
                    jd = dsent
                    otd = outp.tile([128, b], BF16, tag="ot")
                    nc.vector.tensor_tensor(
                        out=otd[:, 0:1024], in0=stg[:, jd, 0:1024],
                        in1=renb[:, 0:1024], op=ALU.mult,
                    )
                    nc.vector.tensor_tensor(
                        out=otd[:, 1024:2048], in0=stg[:, jd, 1024:2048],
                        in1=renb[:, 1024:2048], op=ALU.mult,
                    )
                    nc.sync.dma_start(
                        out=outv[jd * 128:(jd + 1) * 128, :], in_=otd[:]
                    )
                    dsent += 1

                if j >= 9 and (j - 9) % 5 == 0 and wsel_done < mb:
                    phase_wsel(wsel_done)
                    wsel_done += 1
                if j == 88:
                    phi_block()

    nc.compile()
    return nc


_CACHE = {}


def _get_program():
    if "nc" not in _CACHE:
        _CACHE["nc"] = build_program()
    return _CACHE["nc"]


def make_in_maps(embeddings, labels, weight):
    emb = np.asarray(embeddings, dtype=np.float32)
    w = np.asarray(weight, dtype=np.float32)
    labels_np = np.asarray(labels).astype(np.int64)
    emb_bf = np.ascontiguousarray(emb.astype(NPBF))
    embt_bf = np.ascontiguousarray(emb_bf.T)
    w_bf = w.astype(NPBF)
    ident = np.eye(128, dtype=np.float32)
    dsel = np.zeros((16, B), NPBF)
    for k in range(16):
        dsel[k, k * 128:(k + 1) * 128] = 1.0
    in_maps = []
    for k in range(N_CORES):
        wn = np.zeros((CSP, D), NPBF)
        wn[:CS] = w_bf[k * CS:(k + 1) * CS]
        wT = np.ascontiguousarray(wn.T)
        own = (labels_np // CS) == k
        col = labels_np - k * CS
        goff = np.where(own, col, OOB).astype(np.int64)
        goff_arr = np.ascontiguousarray(
            goff.reshape(B // 128, 128).T.astype(np.int32)
        )
        in_maps.append(
            {"emb": emb_bf, "embt": embt_bf, "wt": wT, "wn": wn,
             "goff": goff_arr, "ident": ident, "dsel": dsel}
        )
    return in_maps


def _gather(results, labels):
    labels_np = np.asarray(labels).astype(np.int64)
    bidx = np.arange(B)
    fullT = np.empty((C, B), np.float32)
    for k in range(N_CORES):
        shard = np.asarray(results[k]["out"]).reshape(CSP, B)
        fullT[k * CS:(k + 1) * CS] = shard[:CS]
        # place the device-computed 64*phi values at the target positions
        tv = np.asarray(results[k]["tv"])  # [128, mb]
        own = (labels_np // CS) == k
        ob = bidx[own]
        fullT[labels_np[ob], ob] = tv[ob % 128, ob // 128]
    return fullT.T


def kernel(embeddings, labels, weight):
    nc = _get_program()
    in_maps = make_in_maps(embeddings, labels, weight)
    res = run_bass_kernel_spmd(nc, in_maps, core_ids=list(range(N_CORES)))
    return _gather(res.results, labels)


def kernel_profiled(embeddings, labels, weight, **kw):
    """Like kernel() but also returns the BassKernelResults (exec_time_ns)."""
    nc = _get_program()
    in_maps = make_in_maps(embeddings, labels, weight)
    res = run_bass_kernel_spmd(
        nc, in_maps, core_ids=list(range(N_CORES)), trace=True, **kw
    )
    return _gather(res.results, labels), res
